# revision 1
# baseline (speedup 1.0000x reference)
"""MHSA Trainium2 kernel: B=4, S=2048, D=1024, H=16 heads of 64.

Sharding (8 cores): core c -> batch b=c//2, head-group g=c%2 (8 heads = 512
proj cols). Identical program on every core; only the data differs.

Per-core layouts (host pre-transposes, so no on-device transposes at all):
  xT  [1024, 2048] = x[b].T          wqT/wkT/wvT [1024, 512] = W[cols].T
  woT [512, 1024]  = Wo[:, cols].T   out [2048, 1024] partial (host sums pairs)

Device program:
  qT = wqT.T @ xT + bq   (1/8 score scale pre-folded into wqT/bq on host)
  kT = wkT.T @ xT + bk
  v  = xT.T @ wvT        (v bias deferred: bv @ woT added on host)
  per head h: sT = kT[h].T @ qT[h]; P = exp(sT)  [keys on partitions]
    PV with ones-augmented V: out[65, s] = [V_h | 1].T @ P  -> row 64 = sums
    attnT[h] = PV[0:64] * (1/sums broadcast)
  out_partial = attnT.T @ woT    (DMA'd straight from PSUM)
"""

import os
from contextlib import ExitStack

import numpy as np

import concourse.bass as bass
import concourse.mybir as mybir


def _install_ntff_shim():
    """The agent image's `antenv` lacks `axon_hooks`, which
    run_bass_kernel_spmd imports when trace=True under axon. Provide it,
    wired to the ctypes NTFF hook from trn_agent_boot when available."""
    import sys
    import types
    try:
        from antenv import axon_hooks  # noqa: F401
        return
    except ImportError:
        pass
    try:
        mod = types.ModuleType("antenv.axon_hooks")
        mod._hook = None
        mod.set_axon_ntff_profile_hook = lambda h: setattr(mod, "_hook", h)
        mod.get_axon_ntff_profile_hook = lambda: mod._hook
        import antenv
        sys.modules["antenv.axon_hooks"] = mod
        antenv.axon_hooks = mod
        try:
            from trn_agent_boot.trn_boot import _ntff_profile_via_ctypes
            import os.path
            so = "/opt/axon/libaxon_pjrt.so"
            if os.path.exists(so):
                mod._hook = _ntff_profile_via_ctypes(so)
        except Exception:
            pass
    except Exception:
        pass


_install_ntff_shim()
import concourse.tile as tile
from concourse import bacc
from concourse.bass_utils import run_bass_kernel_spmd

F32 = mybir.dt.float32
F32R = mybir.dt.float32r

S = 2048       # sequence (rows per core's batch)
DF = 1024      # full model dim (contraction for projections)
J = 512        # proj cols per core (8 heads x 64)
HEADS = 8
HD = 64
N_CORES = 8

LAST_RESULT = {}


def _mm(nc, out, lhsT, rhs, mm_dt, **kw):
    nc.tensor.matmul(out, lhsT, rhs, **kw)


def _build(mm_dt):
    MDT = F32 if mm_dt is None else mm_dt
    nc = bacc.Bacc(None, target_bir_lowering=False, debug=False)

    xT_d = nc.declare_dram_parameter("xT", [DF, S], MDT, False)
    wqT_d = nc.declare_dram_parameter("wqT", [DF, J], MDT, False)
    wkT_d = nc.declare_dram_parameter("wkT", [DF, J], MDT, False)
    wvT_d = nc.declare_dram_parameter("wvT", [DF, J], MDT, False)
    bq_d = nc.declare_dram_parameter("bq", [J], F32, False)
    bk_d = nc.declare_dram_parameter("bk", [J], F32, False)
    woT_d = nc.declare_dram_parameter("woT", [J, DF], MDT, False)
    ones_d = nc.declare_dram_parameter("ones", [128, HEADS], MDT, False)
    out_d = nc.declare_dram_parameter("out", [S, DF], F32, isOutput=True)

    with tile.TileContext(nc) as tc, ExitStack() as ctx:
        persist = ctx.enter_context(tc.tile_pool(name="persist", bufs=1))
        qT = [persist.tile([128, S], MDT, name=f"qT{i}", tag=f"qT{i}") for i in range(4)]
        kT = [persist.tile([128, S], MDT, name=f"kT{i}", tag=f"kT{i}") for i in range(4)]
        vt = [persist.tile([128, HEADS, HD + 1], MDT, name=f"v{i}", tag=f"v{i}")
              for i in range(16)]
        bq_sb = persist.tile([128, 4], F32, name="bq", tag="bq")
        bk_sb = persist.tile([128, 4], F32, name="bk", tag="bk")
        nc.sync.dma_start(out=bq_sb, in_=bq_d[:].rearrange("(a p) -> p a", p=128))
        nc.sync.dma_start(out=bk_sb, in_=bk_d[:].rearrange("(a p) -> p a", p=128))

        # ---- Phase A: projections. kT+qT pass, then v pass (wv reuses wk
        # slots). x streamed twice with 2-chunk rotation.
        with tc.tile_pool(name="wq", bufs=1) as wqp, \
             tc.tile_pool(name="wkv", bufs=1) as wkvp, \
             tc.tile_pool(name="xs", bufs=1) as xs, \
             tc.tile_pool(name="psA", bufs=3, space="PSUM") as psA:
            wq_sb = [wqp.tile([128, J], MDT, name=f"wq{k}", tag=f"wq{k}")
                     for k in range(8)]
            wk_sb = [wkvp.tile([128, J], MDT, name=f"wk{k}", tag=f"wkv{k}")
                     for k in range(8)]
            for k in range(8):
                nc.sync.dma_start(out=wk_sb[k], in_=wkT_d[128 * k:128 * (k + 1), :])
                nc.sync.dma_start(out=wq_sb[k], in_=wqT_d[128 * k:128 * (k + 1), :])

            def load_x(sc):
                ts = []
                for kc in range(8):
                    t = xs.tile([128, 512], MDT, name=f"xt{sc % 2}_{kc}",
                                tag=f"xt{sc % 2}_{kc}")
                    nc.sync.dma_start(
                        out=t,
                        in_=xT_d[128 * kc:128 * (kc + 1), 512 * sc:512 * (sc + 1)])
                    ts.append(t)
                return ts

            for sc in range(4):
                ss = slice(512 * sc, 512 * (sc + 1))
                xt = load_x(sc)
                for jt in range(4):
                    jj = slice(128 * jt, 128 * (jt + 1))
                    ps = psA.tile([128, 512], F32, name="psA", tag="psA")
                    for kc in range(8):
                        _mm(nc, ps, wk_sb[kc][:, jj], xt[kc], mm_dt,
                            start=(kc == 0), stop=(kc == 7))
                    nc.vector.tensor_scalar_add(kT[jt][:, ss], ps, bk_sb[:, jt:jt + 1])
                    ps = psA.tile([128, 512], F32, name="psA", tag="psA")
                    for kc in range(8):
                        _mm(nc, ps, wq_sb[kc][:, jj], xt[kc], mm_dt,
                            start=(kc == 0), stop=(kc == 7))
                    nc.vector.tensor_scalar_add(qT[jt][:, ss], ps, bq_sb[:, jt:jt + 1])
            # v pass: wv overwrites wk slots (same tags -> WAR-synced)
            wv_sb = [wkvp.tile([128, J], MDT, name=f"wv{k}", tag=f"wkv{k}")
                     for k in range(8)]
            for k in range(8):
                nc.sync.dma_start(out=wv_sb[k], in_=wvT_d[128 * k:128 * (k + 1), :])
            for sc in range(4):
                xt = load_x(sc)
                for stl in range(4):
                    st = 4 * sc + stl
                    ps = psA.tile([128, 512], F32, name="psA", tag="psA")
                    for kc in range(8):
                        _mm(nc, ps, xt[kc][:, 128 * stl:128 * (stl + 1)], wv_sb[kc],
                            mm_dt, start=(kc == 0), stop=(kc == 7))
                    nc.vector.tensor_copy(
                        vt[st][:, :, 0:HD], ps[:].rearrange("p (h d) -> p h d", h=HEADS))
                    nc.sync.dma_start(
                        out=vt[st][:, :, HD:HD + 1],
                        in_=ones_d[:].rearrange("p (a b) -> p a b", b=1))

        # ---- Phase B: attention; out-proj of sc2=0 interleaved into sc2=1
        # heads as PE filler; batched reciprocal per sc2 off the hot path.
        with tc.tile_pool(name="wo", bufs=1) as wo, \
             tc.tile_pool(name="attn", bufs=1) as attnp, \
             tc.tile_pool(name="pt", bufs=3) as ptp, \
             tc.tile_pool(name="rbc", bufs=2) as rbcp, \
             tc.tile_pool(name="tmp", bufs=3) as tmpp, \
             tc.tile_pool(name="dscr", bufs=2, space="DRAM") as dscr, \
             tc.tile_pool(name="psS", bufs=2, space="PSUM") as psS, \
             tc.tile_pool(name="psPV", bufs=1, space="PSUM") as psPV:
            wo_sb = [wo.tile([128, DF], MDT, name=f"wo{i}", tag=f"wo{i}")
                     for i in range(4)]
            for i in range(4):
                nc.sync.dma_start(out=wo_sb[i], in_=woT_d[128 * i:128 * (i + 1), :])
            attnT = [attnp.tile([128, S], MDT, name=f"at{i}", tag=f"at{i}")
                     for i in range(4)]

            def emit_oproj(st, oc):
                sl = slice(128 * st, 128 * (st + 1))
                ocs = slice(512 * oc, 512 * (oc + 1))
                ps = psS.tile([128, 512], F32, name="fill", tag="fill")
                for jc in range(4):
                    _mm(nc, ps, attnT[jc][:, sl], wo_sb[jc][:, ocs],
                        mm_dt, start=(jc == 0), stop=(jc == 3))
                o_sb = tmpp.tile([128, 512], F32, name="osb", tag="osb")
                nc.vector.tensor_copy(o_sb, ps)
                nc.sync.dma_start(out=out_d[sl, ocs], in_=o_sb)

            for sc2 in range(2):
                s0 = 1024 * sc2
                rd = dscr.tile([8, 1024], F32, name=f"rd{sc2}", tag="rd")
                filler = []
                if sc2 == 1:
                    filler = [(st, oc) for st in range(8) for oc in range(2)]
                for h in range(HEADS):
                    jt, ro = h // 2, 64 * (h % 2)
                    rows = slice(ro, ro + 64)
                    pv_ps = psPV.tile([65, 1024], F32, name="pv", tag="pv")
                    for kt in range(16):
                        tt = slice(128 * kt, 128 * (kt + 1))
                        s_ps = psS.tile([128, 1024], F32, name="sps", tag="sps")
                        _mm(nc, s_ps[:, 0:512], kT[jt][rows, tt],
                            qT[jt][rows, s0:s0 + 512], mm_dt)
                        _mm(nc, s_ps[:, 512:1024], kT[jt][rows, tt],
                            qT[jt][rows, s0 + 512:s0 + 1024], mm_dt)
                        pt = ptp.tile([128, 1024], MDT, name="pt", tag="pt")
                        nc.scalar.activation(pt, s_ps, mybir.ActivationFunctionType.Exp)
                        _mm(nc, pv_ps[:, 0:512], vt[kt][:, h, :], pt[:, 0:512],
                            mm_dt, start=(kt == 0), stop=(kt == 15))
                        _mm(nc, pv_ps[:, 512:1024], vt[kt][:, h, :], pt[:, 512:1024],
                            mm_dt, start=(kt == 0), stop=(kt == 15))
                    # quick pv eviction: raw attnT slice + sums row
                    nc.vector.tensor_copy(attnT[jt][ro:ro + 64, s0:s0 + 1024],
                                          pv_ps[0:64, :])
                    srow = tmpp.tile([1, 1024], F32, name="srow", tag="srow")
                    nc.vector.tensor_copy(srow, pv_ps[64:65, :])
                    nc.sync.dma_start(out=rd[h:h + 1, :], in_=srow)
                    if h >= 1:
                        for _ in range(2):
                            if filler:
                                st, oc = filler.pop(0)
                                emit_oproj(st, oc)
                for st, oc in filler:
                    emit_oproj(st, oc)
                # batched normalization for this sc2
                srows = rbcp.tile([8, 1024], F32, name=f"srows{sc2}", tag="srows")
                nc.sync.dma_start(out=srows, in_=rd[:, :])
                rrec = rbcp.tile([8, 1024], F32, name=f"rrec{sc2}", tag="rrec")
                nc.vector.reciprocal(rrec, srows)
                rd2 = dscr.tile([8, 1024], F32, name=f"rd2{sc2}", tag="rd2")
                nc.sync.dma_start(out=rd2, in_=rrec)
                for jt in range(4):
                    rec = rbcp.tile([128, 1024], F32, name="rec", tag="rec")
                    nc.sync.dma_start(
                        out=rec[0:64, :],
                        in_=rd2[2 * jt:2 * jt + 1, :].partition_broadcast(64))
                    nc.sync.dma_start(
                        out=rec[64:128, :],
                        in_=rd2[2 * jt + 1:2 * jt + 2, :].partition_broadcast(64))
                    nc.vector.tensor_mul(attnT[jt][:, s0:s0 + 1024],
                                         attnT[jt][:, s0:s0 + 1024], rec)
            for st in range(8, 16):
                for oc in range(2):
                    emit_oproj(st, oc)
    nc.compile()
    return nc


_NC_CACHE = {}


def _get_nc(mm_dt):
    key = str(mm_dt)
    if key not in _NC_CACHE:
        _NC_CACHE[key] = _build(mm_dt)
    return _NC_CACHE[key]


def kernel(**inputs):
    x = np.asarray(inputs["x"], np.float32)
    Wq = np.asarray(inputs["Wq"], np.float32)
    bq = np.asarray(inputs["bq"], np.float32)
    Wk = np.asarray(inputs["Wk"], np.float32)
    bk = np.asarray(inputs["bk"], np.float32)
    Wv = np.asarray(inputs["Wv"], np.float32)
    bv = np.asarray(inputs["bv"], np.float32)
    Wo = np.asarray(inputs["Wo"], np.float32)
    bo = np.asarray(inputs["bo"], np.float32)

    scale = np.float32(1.0 / np.sqrt(HD))
    mm_dt = {"f32": None, "f32r": F32R}[os.environ.get("BASS_MM_DT", "f32r")]
    nc = _get_nc(mm_dt)

    in_maps = []
    bvwo = []     # host-side bv @ woT rows, one per core
    for c in range(N_CORES):
        b, g = c // 2, c % 2
        cols = slice(J * g, J * (g + 1))
        woTs = np.ascontiguousarray(Wo[:, cols].T)
        in_maps.append({
            "xT": np.ascontiguousarray(x[b].T),
            "wqT": np.ascontiguousarray(Wq[cols, :].T) * scale,
            "wkT": np.ascontiguousarray(Wk[cols, :].T),
            "wvT": np.ascontiguousarray(Wv[cols, :].T),
            "bq": np.ascontiguousarray(bq[cols]) * scale,
            "bk": np.ascontiguousarray(bk[cols]),
            "woT": woTs,
            "ones": np.ones((128, HEADS), np.float32),
            "out": np.zeros((S, DF), np.float32),
        })
        bvwo.append(bv[cols] @ woTs)
    for m in in_maps:
        m.pop("out")

    res = run_bass_kernel_spmd(
        nc, in_maps, list(range(N_CORES)),
        trace=bool(os.environ.get("BASS_TRACE")))
    LAST_RESULT["exec_time_ns"] = res.exec_time_ns
    LAST_RESULT["mean_exec_time_ns"] = getattr(res, "mean_exec_time_ns", None)
    LAST_RESULT["profile_json"] = res.profile_json
    it = res.instructions_and_trace
    LAST_RESULT["trace_path"] = it[1] if it else None
    LAST_RESULT["insts"] = it[0] if it else None

    B = x.shape[0]
    out = np.empty((B, S, DF), np.float32)
    for b in range(B):
        out[b] = (res.results[2 * b]["out"] + res.results[2 * b + 1]["out"]
                  + bvwo[2 * b][None, :] + bvwo[2 * b + 1][None, :]
                  + bo[None, :])
    return out



# revision 10
# speedup vs baseline: 1.2696x; 1.2696x over previous
"""MHSA Trainium2 kernel: B=4, S=2048, D=1024, H=16 heads of 64.

Sharding (8 cores): core c -> batch b=c//2, head-group g=c%2 (8 heads = 512
proj cols). Identical program on every core; only the data differs.

All tensors bf16 (PSUM accumulation f32). The schedule is built around the
PE DVFS ramp: any PE stall drops the clock 2.4->1.2 GHz for ~3us, so the
program is one continuous PE stream. Attention runs as a rolling slot
pipeline over m = h*16 + kt:

  slot m: scores(m) [2 matmuls] -> PV(m-2) [2 matmuls] -> filler matmuls

with exp(m) on the Scalar engine trailing scores(m) and leading PV(m) by
two slots (psS bufs=2, pt bufs=4 decouple the engines). Fillers are the
remaining projections (kT jt1-3, qT, vT tail) and the out-projection,
drip-fed so the PE always has independent work while ACT catches up.

Per-core layouts (host pre-transposes; no on-device transposes):
  xT  [1024, 2048] = x[b].T          wqT/wkT/wvT [1024, 512] = W[cols].T
  woT [512, 1024]  = Wo[:, cols].T   out [2048, 1024] partial (host sums)

  qT = wqT.T @ xT + bq   (1/8 score scale pre-folded into wqT/bq on host)
  kT = wkT.T @ xT + bk
  v  = xT.T @ wvT        (v bias deferred: bv @ woT added on host)
  per head h: sT = kT[h].T @ qT[h]; P = exp(sT)  [keys on partitions]
    PV with ones-augmented V: pv[65, s] = [V_h | 1].T @ P -> row 64 = sums
    attnT[h] = pv[0:64] * (1/sums broadcast)
  out_partial = attnT.T @ woT
"""

import os
from collections import deque
from contextlib import ExitStack

import numpy as np

import concourse.bass as bass
import concourse.mybir as mybir


def _install_ntff_shim():
    """The agent image's `antenv` lacks `axon_hooks`, which
    run_bass_kernel_spmd imports when trace=True under axon. Provide it,
    wired to the ctypes NTFF hook from trn_agent_boot when available."""
    import sys
    import types
    try:
        from antenv import axon_hooks  # noqa: F401
        return
    except ImportError:
        pass
    try:
        mod = types.ModuleType("antenv.axon_hooks")
        mod._hook = None
        mod.set_axon_ntff_profile_hook = lambda h: setattr(mod, "_hook", h)
        mod.get_axon_ntff_profile_hook = lambda: mod._hook
        import antenv
        sys.modules["antenv.axon_hooks"] = mod
        antenv.axon_hooks = mod
        try:
            from trn_agent_boot.trn_boot import _ntff_profile_via_ctypes
            import os.path
            so = "/opt/axon/libaxon_pjrt.so"
            if os.path.exists(so):
                mod._hook = _ntff_profile_via_ctypes(so)
        except Exception:
            pass
    except Exception:
        pass


_install_ntff_shim()
import concourse.tile as tile
from concourse import bacc
from concourse.bass_utils import run_bass_kernel_spmd

F32 = mybir.dt.float32
BF16 = mybir.dt.bfloat16

S = 2048       # sequence (rows per core's batch)
DF = 1024      # full model dim (contraction for projections)
J = 512        # proj cols per core (8 heads x 64)
HEADS = 8
HD = 64
KC = 8         # 128-row contraction chunks of DF
N_CORES = 8

LAST_RESULT = {}


def _build():
    nc = bacc.Bacc(None, target_bir_lowering=False, debug=False)

    xT_d = nc.declare_dram_parameter("xT", [DF, S], BF16, False)
    wqT_d = nc.declare_dram_parameter("wqT", [DF, J], BF16, False)
    wkT_d = nc.declare_dram_parameter("wkT", [DF, J], BF16, False)
    wvT_d = nc.declare_dram_parameter("wvT", [DF, J], BF16, False)
    bq_d = nc.declare_dram_parameter("bq", [J], F32, False)
    bk_d = nc.declare_dram_parameter("bk", [J], F32, False)
    woT_d = nc.declare_dram_parameter("woT", [J, DF], BF16, False)
    ones_d = nc.declare_dram_parameter("ones", [128, HEADS], BF16, False)
    out_d = nc.declare_dram_parameter("out", [S, DF], F32, isOutput=True)

    with tile.TileContext(nc) as tc, ExitStack() as ctx:
        persist = ctx.enter_context(tc.tile_pool(name="persist", bufs=1))
        # PSUM budget (8 banks of 2KB/partition):
        #   psS 2x[128,1024]f32 = 4, psPV 1x[65,1024]f32 = 2, psA 2x[128,512] = 2
        psS = ctx.enter_context(tc.tile_pool(name="psS", bufs=2, space="PSUM"))
        psPV = ctx.enter_context(tc.tile_pool(name="psPV", bufs=1, space="PSUM"))
        psA = ctx.enter_context(tc.tile_pool(name="psA", bufs=2, space="PSUM"))
        ptp = ctx.enter_context(tc.tile_pool(name="ptp", bufs=4))
        tmpp = ctx.enter_context(tc.tile_pool(name="tmp", bufs=3))
        rbcp = ctx.enter_context(tc.tile_pool(name="rbc", bufs=2))
        dscr = ctx.enter_context(tc.tile_pool(name="dscr", bufs=2, space="DRAM"))

        x_sb = [persist.tile([128, S], BF16, name=f"x{k}", tag=f"x{k}")
                for k in range(KC)]
        wk_sb = [persist.tile([128, J], BF16, name=f"wk{k}", tag=f"wk{k}")
                 for k in range(KC)]
        wv_sb = [persist.tile([128, J], BF16, name=f"wv{k}", tag=f"wv{k}")
                 for k in range(KC)]
        wq_sb = [persist.tile([128, J], BF16, name=f"wq{k}", tag=f"wq{k}")
                 for k in range(KC)]
        wo_sb = [persist.tile([128, DF], BF16, name=f"wo{i}", tag=f"wo{i}")
                 for i in range(4)]
        kT = [persist.tile([128, S], BF16, name=f"kT{i}", tag=f"kT{i}")
              for i in range(4)]
        qT = [persist.tile([128, S], BF16, name=f"qT{i}", tag=f"qT{i}")
              for i in range(4)]
        attnT = [persist.tile([128, S], BF16, name=f"at{i}", tag=f"at{i}")
                 for i in range(4)]
        vt = [persist.tile([128, HEADS, HD + 1], BF16, name=f"v{i}", tag=f"v{i}")
              for i in range(16)]
        bq_sb = persist.tile([128, 4], F32, name="bq", tag="bq")
        bk_sb = persist.tile([128, 4], F32, name="bk", tag="bk")
        ones_sb = persist.tile([128, HEADS, 1], BF16, name="ones", tag="ones")

        # DMA order = sync-queue issue order; wk/x interleaved so the first
        # kT matmuls start as early as possible.
        for k in range(KC):
            nc.sync.dma_start(out=wk_sb[k], in_=wkT_d[128 * k:128 * (k + 1), :])
            nc.sync.dma_start(out=x_sb[k], in_=xT_d[128 * k:128 * (k + 1), :])
        nc.sync.dma_start(out=bk_sb, in_=bk_d[:].rearrange("(a p) -> p a", p=128))
        nc.sync.dma_start(out=bq_sb, in_=bq_d[:].rearrange("(a p) -> p a", p=128))
        nc.sync.dma_start(out=ones_sb, in_=ones_d[:].rearrange("p (a b) -> p a b", b=1))
        for k in range(KC):
            nc.sync.dma_start(out=wv_sb[k], in_=wvT_d[128 * k:128 * (k + 1), :])
        for k in range(KC):
            nc.sync.dma_start(out=wq_sb[k], in_=wqT_d[128 * k:128 * (k + 1), :])
        for i in range(4):
            nc.sync.dma_start(out=wo_sb[i], in_=woT_d[128 * i:128 * (i + 1), :])

        # ---- filler machinery: each closure emits ONE PE matmul (plus the
        # group's eviction op on its last member).
        fills = deque()

        def emit_fill(n):
            for _ in range(n):
                if not fills:
                    return
                fills.popleft()()

        def kq_group(dst, w_sb, b_sb, jt_i, sc):
            jj = slice(128 * jt_i, 128 * (jt_i + 1))
            ss = slice(512 * sc, 512 * (sc + 1))
            st = {}

            def mk(kc):
                def f():
                    if kc == 0:
                        st['ps'] = psA.tile([128, 512], F32, name="psA", tag="psA")
                    nc.tensor.matmul(st['ps'], w_sb[kc][:, jj], x_sb[kc][:, ss],
                                     start=(kc == 0), stop=(kc == 7))
                    if kc == 7:
                        nc.vector.tensor_scalar_add(
                            dst[jt_i][:, ss], st['ps'], b_sb[:, jt_i:jt_i + 1])
                return f
            return [mk(kc) for kc in range(KC)]

        def vt_group(st_i):
            st = {}

            def mk(kc):
                def f():
                    if kc == 0:
                        st['ps'] = psA.tile([128, 512], F32, name="psA", tag="psA")
                    nc.tensor.matmul(
                        st['ps'], x_sb[kc][:, 128 * st_i:128 * (st_i + 1)],
                        wv_sb[kc], start=(kc == 0), stop=(kc == 7))
                    if kc == 7:
                        nc.vector.tensor_copy(
                            vt[st_i][:, :, 0:HD],
                            st['ps'][:].rearrange("p (h d) -> p h d", h=HEADS))
                        nc.vector.tensor_copy(vt[st_i][:, :, HD:HD + 1], ones_sb)
                return f
            return [mk(kc) for kc in range(KC)]

        def oproj_group(st_i, oc):
            sl = slice(128 * st_i, 128 * (st_i + 1))
            ocs = slice(512 * oc, 512 * (oc + 1))
            st = {}

            def mk(jc):
                def f():
                    if jc == 0:
                        st['ps'] = psA.tile([128, 512], F32, name="psA", tag="psA")
                    nc.tensor.matmul(st['ps'], attnT[jc][:, sl], wo_sb[jc][:, ocs],
                                     start=(jc == 0), stop=(jc == 3))
                    if jc == 3:
                        o_sb = tmpp.tile([128, 512], F32, name="osb", tag="osb")
                        nc.vector.tensor_copy(o_sb, st['ps'])
                        nc.sync.dma_start(out=out_d[sl, ocs], in_=o_sb)
                return f
            return [mk(jc) for jc in range(4)]

        def run_group(ops):
            for f in ops:
                f()

        # ---- upfront PE work (ACT idle here; keep it minimal):
        # kT jt0, vt[0..11], qT jt0 c0. vt[12..15] lead the c0 fill queue at
        # 3 fills/slot so vt[12+i] lands well before its PV at slot 13+i.
        for sc in range(4):
            run_group(kq_group(kT, wk_sb, bk_sb, 0, sc))
        for st_i in range(12):
            run_group(vt_group(st_i))
        for sc in range(2):
            run_group(kq_group(qT, wq_sb, bq_sb, 0, sc))

        for st_i in range(12, 16):
            fills.extend(vt_group(st_i))
        for jt_i in (1, 2, 3):
            for sc in range(4):
                fills.extend(kq_group(kT, wk_sb, bk_sb, jt_i, sc))
            for sc in range(2):
                fills.extend(kq_group(qT, wq_sb, bq_sb, jt_i, sc))
        for sc in (2, 3):
            fills.extend(kq_group(qT, wq_sb, bq_sb, 0, sc))

        def attention_pass(c2):
            s0 = 1024 * c2
            rd = dscr.tile([8, 1024], F32, name=f"rd{c2}", tag="rd")
            pend = deque()
            state = {}

            def emit_pv(ent):
                h, kt, ptt = ent
                jt, ro = h // 2, 64 * (h % 2)
                if kt == 0:
                    state['pv'] = psPV.tile([65, 1024], F32, name="pv", tag="pv")
                pv = state['pv']
                nc.tensor.matmul(pv[:, 0:512], vt[kt][:, h, :], ptt[:, 0:512],
                                 start=(kt == 0), stop=(kt == 15))
                nc.tensor.matmul(pv[:, 512:1024], vt[kt][:, h, :], ptt[:, 512:1024],
                                 start=(kt == 0), stop=(kt == 15))
                if kt == 15:
                    # one copy frees the PSUM accumulator; head block and
                    # sums row then leave from the SBUF staging tile
                    stage = tmpp.tile([65, 1024], F32, name="pvs", tag="pvs")
                    nc.vector.tensor_copy(stage, pv[0:65, :])
                    nc.vector.tensor_copy(attnT[jt][ro:ro + 64, s0:s0 + 1024],
                                          stage[0:64, :])
                    nc.sync.dma_start(out=rd[h:h + 1, :], in_=stage[64:65, :])

            for h in range(HEADS):
                jt, ro = h // 2, 64 * (h % 2)
                rows = slice(ro, ro + 64)
                for kt in range(16):
                    m = 16 * h + kt
                    tt = slice(128 * kt, 128 * (kt + 1))
                    sps = psS.tile([128, 1024], F32, name="sps", tag="sps")
                    nc.tensor.matmul(sps[:, 0:512], kT[jt][rows, tt],
                                     qT[jt][rows, s0:s0 + 512])
                    nc.tensor.matmul(sps[:, 512:1024], kT[jt][rows, tt],
                                     qT[jt][rows, s0 + 512:s0 + 1024])
                    if kt == 2 and h > 0:
                        emit_fill(3)  # cover pv eviction before PV(h, 0)
                    if len(pend) >= 2:
                        emit_pv(pend.popleft())
                    nf = 3 if (c2 == 0 and m < 12) else (2 if c2 == 0 else 1)
                    emit_fill(nf)
                    ptt = ptp.tile([128, 1024], BF16, name="pt", tag="pt")
                    nc.scalar.activation(ptt, sps, mybir.ActivationFunctionType.Exp)
                    pend.append((h, kt, ptt))
            while pend:
                emit_pv(pend.popleft())
                emit_fill(2)
            return rd

        def norm(c2, rd):
            s0 = 1024 * c2
            srows = rbcp.tile([8, 1024], F32, name=f"srows{c2}", tag="srows")
            nc.sync.dma_start(out=srows, in_=rd[:, :])
            rrec = rbcp.tile([8, 1024], F32, name=f"rrec{c2}", tag="rrec")
            nc.vector.reciprocal(rrec, srows)
            rd2 = dscr.tile([8, 1024], F32, name=f"rd2{c2}", tag="rd2")
            nc.sync.dma_start(out=rd2, in_=rrec)
            for jt2 in range(4):
                rec = rbcp.tile([128, 1024], F32, name="rec", tag="rec")
                nc.sync.dma_start(
                    out=rec[0:64, :],
                    in_=rd2[2 * jt2:2 * jt2 + 1, :].partition_broadcast(64))
                nc.sync.dma_start(
                    out=rec[64:128, :],
                    in_=rd2[2 * jt2 + 1:2 * jt2 + 2, :].partition_broadcast(64))
                nc.vector.tensor_mul(attnT[jt2][:, s0:s0 + 1024],
                                     attnT[jt2][:, s0:s0 + 1024], rec)

        rd0 = attention_pass(0)
        norm(0, rd0)
        # c1 fills: qT c1 for jt1-3 first (needed by c1 heads 2+), then the
        # out-projection of the c0 rows (legal once norm(0) lands).
        for jt_i in (1, 2, 3):
            for sc in (2, 3):
                fills.extend(kq_group(qT, wq_sb, bq_sb, jt_i, sc))
        for st_i in range(8):
            for oc in range(2):
                fills.extend(oproj_group(st_i, oc))
        rd1 = attention_pass(1)
        norm(1, rd1)
        # tail: drain leftover fills, then out-projection of the c1 rows
        emit_fill(len(fills))
        for st_i in range(8, 16):
            for oc in range(2):
                run_group(oproj_group(st_i, oc))
    nc.compile()
    return nc


_NC_CACHE = {}


def _get_nc():
    if "nc" not in _NC_CACHE:
        _NC_CACHE["nc"] = _build()
    return _NC_CACHE["nc"]


def kernel(**inputs):
    from ml_dtypes import bfloat16 as bf16

    x = np.asarray(inputs["x"], np.float32)
    Wq = np.asarray(inputs["Wq"], np.float32)
    bq = np.asarray(inputs["bq"], np.float32)
    Wk = np.asarray(inputs["Wk"], np.float32)
    bk = np.asarray(inputs["bk"], np.float32)
    Wv = np.asarray(inputs["Wv"], np.float32)
    bv = np.asarray(inputs["bv"], np.float32)
    Wo = np.asarray(inputs["Wo"], np.float32)
    bo = np.asarray(inputs["bo"], np.float32)

    scale = np.float32(1.0 / np.sqrt(HD))
    nc = _get_nc()

    in_maps = []
    bvwo = []     # host-side bv @ woT rows, one per core
    for c in range(N_CORES):
        b, g = c // 2, c % 2
        cols = slice(J * g, J * (g + 1))
        woTs = np.ascontiguousarray(Wo[:, cols].T)
        in_maps.append({
            "xT": np.ascontiguousarray(x[b].T).astype(bf16),
            "wqT": (np.ascontiguousarray(Wq[cols, :].T) * scale).astype(bf16),
            "wkT": np.ascontiguousarray(Wk[cols, :].T).astype(bf16),
            "wvT": np.ascontiguousarray(Wv[cols, :].T).astype(bf16),
            "bq": np.ascontiguousarray(bq[cols]) * scale,
            "bk": np.ascontiguousarray(bk[cols]),
            "woT": woTs.astype(bf16),
            "ones": np.ones((128, HEADS), bf16),
        })
        bvwo.append(bv[cols] @ woTs)

    res = run_bass_kernel_spmd(
        nc, in_maps, list(range(N_CORES)),
        trace=bool(os.environ.get("BASS_TRACE")))
    LAST_RESULT["exec_time_ns"] = res.exec_time_ns
    LAST_RESULT["mean_exec_time_ns"] = getattr(res, "mean_exec_time_ns", None)
    LAST_RESULT["profile_json"] = res.profile_json
    it = res.instructions_and_trace
    LAST_RESULT["trace_path"] = it[1] if it else None
    LAST_RESULT["insts"] = it[0] if it else None

    B = x.shape[0]
    out = np.empty((B, S, DF), np.float32)
    for b in range(B):
        out[b] = (res.results[2 * b]["out"] + res.results[2 * b + 1]["out"]
                  + bvwo[2 * b][None, :] + bvwo[2 * b + 1][None, :]
                  + bo[None, :])
    return out
